# revision 1
# baseline (speedup 1.0000x reference)
"""FFT conv block (rfft2 -> per-channel complex multiply -> irfft2) on 8 trn2 cores.

Math (verified vs reference in float64):
  Work per (b,c) image [256, 256], sharded over channels (8 ch/core):
    T1: rfft over H via matmul            Y1[w, kh]  kh in 0..128
    T2: full fft over W via matmul        Y2[kw, kh] kw in 0..255
    wm: Yw = Y2 * W_eff[kw, kh]           (elementwise complex, DVE)
    T3: ifft over W via matmul            Z[kh, w']
    T4: irfft over H via matmul           y[h, w']   real
  W_eff remaps the reference's [kh_full, kw_half] weights onto the
  [kh_half, kw_full] quarter-plane (conj-flip for kw>128; kw in {0,128}
  columns symmetrized over kh).  Matmuls run in float32r (~1.5e-4 rel).
"""
import sys
sys.path.insert(0, "/opt/trn_rl_repo")
import numpy as np

B, C, H, W = 16, 64, 256, 256
KHF = H // 2 + 1          # 129
N_CORES = 8
NC_LOC = C // N_CORES     # 8 channels per core
NB = B                    # 16 batch images per channel
G = 8                     # supergroup size (images of same channel)


def _consts():
    h = np.arange(H)[:, None]
    kh = np.arange(KHF)[None, :]
    th = 2 * np.pi * h * kh / H                      # [H, KHF]
    z = np.zeros((H, 1))
    M1 = np.concatenate([np.cos(th), z, -np.sin(th), z], axis=1)  # [256, 260]

    w_ = np.arange(W)[:, None]
    kw = np.arange(W)[None, :]
    tw = 2 * np.pi * w_ * kw / W                     # [W, W]
    P = np.cos(tw)                                   # [256, 256]
    Q = np.sin(tw)

    kw2 = np.arange(W)[:, None]
    wp = np.arange(W)[None, :]
    t3 = 2 * np.pi * kw2 * wp / W
    C3 = np.cos(t3) / W                              # [256, 256]
    S3 = np.sin(t3) / W

    kh2 = np.arange(KHF)[:, None]
    hp = np.arange(H)[None, :]
    t4 = 2 * np.pi * kh2 * hp / H
    alpha = np.where((kh2 == 0) | (kh2 == H // 2), 1.0, 2.0)
    A4 = alpha * np.cos(t4) / H                      # [129, 256]
    B4 = -alpha * np.sin(t4) / H                     # [129, 256] (rows 0,128 zero)
    B4[0, :] = A4[128, :]   # row 0 slot carries the nyquist contribution via Zi[0]
    f32 = np.float32
    return (M1.astype(f32), P.astype(f32), Q.astype(f32), C3.astype(f32),
            S3.astype(f32), A4.astype(f32), B4.astype(f32))


def _w_eff(wr, wi):
    """wr, wi: [256(kh), 129(kw)] reference layout -> W_eff [129(kh), 256(kw)]."""
    w = wr.astype(np.float64) + 1j * wi.astype(np.float64)
    kh = np.arange(KHF)                              # 0..128
    khc = (H - kh) % H
    eff = np.empty((KHF, W), dtype=np.complex128)
    eff[:, 0:W // 2 + 1] = w[0:KHF, :]
    for kw in range(W // 2 + 1, W):
        eff[:, kw] = np.conj(w[khc, W - kw])
    for col in (0, W // 2):
        eff[:, col] = 0.5 * (w[kh, col] + np.conj(w[khc, col]))
    return eff                                        # [129, 256] complex128


def _np_pipeline(x_img, eff):
    """Golden single-image model (float64) used by test harness."""
    Q_ = np.fft.fft2(x_img.astype(np.float64))[0:KHF, :]
    Yq = Q_ * eff
    return np.fft.irfft(np.fft.ifft(Yq, axis=1), n=H, axis=0)


def build_nc(nc_loc=NC_LOC, nb=NB, g=G, repeat=1):
    import concourse.bass as bass
    import concourse.mybir as mybir
    import concourse.tile as tile
    from concourse import bacc

    f32, f32r = mybir.dt.float32, mybir.dt.float32r
    bf16 = mybir.dt.bfloat16
    KP = 130                               # kh padded to even (fp32r ISA rule)
    n_img = nb * nc_loc

    M1, P, Q, C3, S3, A4, B4 = _consts()

    nc = bacc.Bacc("TRN2", target_bir_lowering=False)
    x_d = nc.dram_tensor("x", [n_img, H, W], f32r, kind="ExternalInput")
    w_d = nc.dram_tensor("w", [nc_loc, 3, W, 2 * KP], f32, kind="ExternalInput")
    y_d = nc.dram_tensor("y", [n_img, H, W], f32, kind="ExternalOutput")

    c_m1 = nc.inline_tensor(M1, "c_m1")            # [256, 260]
    c_p = nc.inline_tensor(P, "c_p")               # [256, 256]
    c_q = nc.inline_tensor(Q, "c_q")
    c_nq = nc.inline_tensor(-Q, "c_nq")
    c_c3 = nc.inline_tensor(C3, "c_c3")
    c_s3 = nc.inline_tensor(S3, "c_s3")
    c_ns3 = nc.inline_tensor(-S3, "c_ns3")
    c_a4m = nc.inline_tensor(A4[0:128], "c_a4m")   # [128, 256]
    c_b4m = nc.inline_tensor(B4[0:128], "c_b4m")   # [128, 256]

    from contextlib import ExitStack
    with tile.TileContext(nc) as tc, ExitStack() as es:
        cpool = es.enter_context(tc.tile_pool(name="const", bufs=1))
        stage = es.enter_context(tc.tile_pool(name="stage", bufs=2))
        wpool = es.enter_context(tc.tile_pool(name="wpool", bufs=1))
        xrp = es.enter_context(tc.tile_pool(name="xr", bufs=2))
        y1p = es.enter_context(tc.tile_pool(name="y1", bufs=4))
        y2wp = es.enter_context(tc.tile_pool(name="y2w", bufs=2))
        wtmp = es.enter_context(tc.tile_pool(name="wtmp", bufs=2))
        nyqbp = es.enter_context(tc.tile_pool(name="nyqb", bufs=2))
        zsb = es.enter_context(tc.tile_pool(name="zsb", bufs=4))
        znsb = es.enter_context(tc.tile_pool(name="znsb", bufs=2))
        ysb = es.enter_context(tc.tile_pool(name="ysb", bufs=3))
        ps1 = es.enter_context(tc.tile_pool(name="ps1", bufs=2, space="PSUM"))
        ps2 = es.enter_context(tc.tile_pool(name="ps2", bufs=2, space="PSUM"))
        ps3 = es.enter_context(tc.tile_pool(name="ps3", bufs=2, space="PSUM"))
        ps4 = es.enter_context(tc.tile_pool(name="ps4", bufs=2, space="PSUM"))
        if True:

            def load_const(src, rows, cols, nm, dt_):
                tiles = []
                nch = (rows + 127) // 128
                for ch in range(nch):
                    r0, r1 = ch * 128, min((ch + 1) * 128, rows)
                    st = stage.tile([r1 - r0, cols], f32, name=f"st_{nm}{ch}")
                    nc.sync.dma_start(out=st, in_=src[r0:r1, :])
                    t = cpool.tile([r1 - r0, cols], dt_, name=f"c_{nm}{ch}")
                    nc.vector.tensor_copy(out=t, in_=st)
                    tiles.append(t)
                return tiles

            m1t = load_const(c_m1, H, 2 * KP, "m1", f32r)      # 2 x [128, 260]
            pt = load_const(c_p, W, W, "p", f32r)              # 2 x [128, 256]
            qt = load_const(c_q, W, W, "q", f32r)
            nqt = load_const(c_nq, W, W, "nq", f32r)
            c3t = load_const(c_c3, W, W, "c3", f32r)
            s3t = load_const(c_s3, W, W, "s3", f32r)
            ns3t = load_const(c_ns3, W, W, "ns3", f32r)
            a4mt = load_const(c_a4m, 128, W, "a4m", f32r)[0]
            b4mt = load_const(c_b4m, 128, W, "b4m", f32r)[0]
            c3tb = load_const(c_c3, W, W, "c3b", bf16)         # bf16 for nyquist mm
            ns3tb = load_const(c_ns3, W, W, "ns3b", bf16)

            # weights: [128(kw), 2(kwchunk), 2(imgdup), KP] fp32 per (c, kind)
            wt = []
            for cl in range(nc_loc):
                row = []
                for k in range(3):
                    t = wpool.tile([128, 2, 2, KP], f32, name=f"w{cl}_{k}")
                    for ch in range(2):
                        nc.sync.dma_start(
                            out=t[:, ch, :, :],
                            in_=w_d[cl, k, ch * 128:(ch + 1) * 128, :].rearrange(
                                "p (i k) -> p i k", i=2),
                        )
                    row.append(t)
                wt.append(row)

            npairs = g // 2
            for _rep in range(repeat):
              for cl in range(nc_loc):
                for sg0 in range(0, nb, g):
                    # ---- phase A: T1 + T2 + wmul per pair ----
                    y2r_sg = y2wp.tile([128, 2, g, KP], f32r, name="y2r_sg")
                    y2i_sg = y2wp.tile([128, 2, g, KP], f32r, name="y2i_sg")
                    for pr in range(npairs):
                        y1t = [y1p.tile([128, 2, 2 * KP], f32r, name=f"y1_{wc}")
                               for wc in range(2)]
                        img0 = cl * nb + sg0 + 2 * pr
                        xr = xrp.tile([128, 2, 2, W], f32r, name="xr")
                        nc.sync.dma_start(
                            out=xr,
                            in_=x_d[img0:img0 + 2].rearrange(
                                "i (c p) w -> p i c w", c=2))
                        for j in range(2):
                            for wc in range(2):
                                t1 = ps1.tile([128, 2 * KP], f32, name="t1ps")
                                nc.tensor.matmul(
                                    t1, xr[:, j, 0, wc * 128:(wc + 1) * 128], m1t[0],
                                    start=True, stop=False)
                                nc.tensor.matmul(
                                    t1, xr[:, j, 1, wc * 128:(wc + 1) * 128], m1t[1],
                                    start=False, stop=True)
                                nc.scalar.copy(out=y1t[wc][:, j, :], in_=t1)
                        # T2: Y2r = P^T Y1r + Q^T Y1i ; Y2i = P^T Y1i - Q^T Y1r
                        for kwc in range(2):
                            cols = slice(kwc * 128, (kwc + 1) * 128)
                            y2r = ps2.tile([128, 2, KP], f32, name="y2rps", bufs=1)
                            y2i = ps2.tile([128, 2, KP], f32, name="y2ips", bufs=1)
                            r_, i_ = slice(0, KP), slice(KP, 2 * KP)
                            nc.tensor.matmul(y2r, pt[0][:, cols], y1t[0][:, :, r_], start=True, stop=False)
                            nc.tensor.matmul(y2r, pt[1][:, cols], y1t[1][:, :, r_], start=False, stop=False)
                            nc.tensor.matmul(y2r, qt[0][:, cols], y1t[0][:, :, i_], start=False, stop=False)
                            nc.tensor.matmul(y2r, qt[1][:, cols], y1t[1][:, :, i_], start=False, stop=True)
                            nc.tensor.matmul(y2i, pt[0][:, cols], y1t[0][:, :, i_], start=True, stop=False)
                            nc.tensor.matmul(y2i, pt[1][:, cols], y1t[1][:, :, i_], start=False, stop=False)
                            nc.tensor.matmul(y2i, nqt[0][:, cols], y1t[0][:, :, r_], start=False, stop=False)
                            nc.tensor.matmul(y2i, nqt[1][:, cols], y1t[1][:, :, r_], start=False, stop=True)
                            # wmul: Ywr = Y2r*Wr + Y2i*(-Wi); Ywi = Y2r*Wi + Y2i*Wr
                            wr_ = wt[cl][0][:, kwc, :, :]
                            wi_ = wt[cl][1][:, kwc, :, :]
                            nwi = wt[cl][2][:, kwc, :, :]
                            sl = slice(2 * pr, 2 * pr + 2)
                            ta = wtmp.tile([128, 2, KP], f32, name="ta")
                            tb = wtmp.tile([128, 2, KP], f32, name="tb")
                            nc.vector.tensor_mul(ta, y2r, wr_)
                            nc.vector.tensor_mul(tb, y2i, nwi)
                            nc.vector.tensor_add(y2r_sg[:, kwc, sl, :], ta, tb)
                            tc_ = wtmp.tile([128, 2, KP], f32, name="tc")
                            td = wtmp.tile([128, 2, KP], f32, name="td")
                            nc.vector.tensor_mul(tc_, y2r, wi_)
                            nc.vector.tensor_mul(td, y2i, wr_)
                            nc.vector.tensor_add(y2i_sg[:, kwc, sl, :], tc_, td)
                    # ---- phase B: nyquist (kh=128) T3, real part only, bf16 mm ----
                    nbr = nyqbp.tile([128, 2, g], bf16, name="nbr")
                    nbi = nyqbp.tile([128, 2, g], bf16, name="nbi")
                    nc.vector.tensor_copy(out=nbr, in_=y2r_sg[:, :, :, 128])
                    nc.vector.tensor_copy(out=nbi, in_=y2i_sg[:, :, :, 128])
                    zn = ps1.tile([g, W], f32, name="znps", tag="t1ps")
                    nc.tensor.matmul(zn, nbr[:, 0, :], c3tb[0], start=True, stop=False)
                    nc.tensor.matmul(zn, nbr[:, 1, :], c3tb[1], start=False, stop=False)
                    nc.tensor.matmul(zn, nbi[:, 0, :], ns3tb[0], start=False, stop=False)
                    nc.tensor.matmul(zn, nbi[:, 1, :], ns3tb[1], start=False, stop=True)
                    znr = znsb.tile([g, W], f32r, name="znr")
                    nc.vector.tensor_copy(out=znr, in_=zn)
                    # ---- phase C: per image T3 main + T4 ----
                    for gg in range(g):
                        img = cl * nb + sg0 + gg
                        zr = ps3.tile([128, W], f32, name="zrps", bufs=1)
                        zi = ps3.tile([128, W], f32, name="zips", bufs=1)
                        kmain = slice(0, 128)
                        nc.tensor.matmul(zr, y2r_sg[:, 0, gg, kmain], c3t[0], start=True, stop=False)
                        nc.tensor.matmul(zr, y2r_sg[:, 1, gg, kmain], c3t[1], start=False, stop=False)
                        nc.tensor.matmul(zr, y2i_sg[:, 0, gg, kmain], ns3t[0], start=False, stop=False)
                        nc.tensor.matmul(zr, y2i_sg[:, 1, gg, kmain], ns3t[1], start=False, stop=True)
                        nc.tensor.matmul(zi, y2i_sg[:, 0, gg, kmain], c3t[0], start=True, stop=False)
                        nc.tensor.matmul(zi, y2i_sg[:, 1, gg, kmain], c3t[1], start=False, stop=False)
                        nc.tensor.matmul(zi, y2r_sg[:, 0, gg, kmain], s3t[0], start=False, stop=False)
                        nc.tensor.matmul(zi, y2r_sg[:, 1, gg, kmain], s3t[1], start=False, stop=True)
                        zrs = zsb.tile([128, W], f32r, name="zrs")
                        zis = zsb.tile([128, W], f32r, name="zis")
                        nc.scalar.copy(out=zrs, in_=zr)
                        nc.vector.tensor_copy(out=zis, in_=zi)
                        nc.sync.dma_start(out=zis[0:1, :], in_=znr[gg:gg + 1, :])
                        yt = ysb.tile([128, 2, W], f32, name="yt")
                        for hc in range(2):
                            cols = slice(hc * 128, (hc + 1) * 128)
                            yp = ps4.tile([128, W], f32, name="yps")
                            nc.tensor.matmul(yp, a4mt[:, cols], zrs, start=True, stop=False)
                            nc.tensor.matmul(yp, b4mt[:, cols], zis, start=False, stop=True)
                            nc.scalar.copy(out=yt[:, hc, :], in_=yp)
                        nc.sync.dma_start(
                            out=y_d[img].rearrange("(c p) w -> p c w", c=2), in_=yt)
    nc.compile()
    return nc


def _prep_weights(w_real, w_imag, core, nc_loc=NC_LOC):
    KP = 130
    warr = np.zeros((nc_loc, 3, W, 2 * KP), np.float32)
    effs = []
    for cl in range(nc_loc):
        eff = _w_eff(w_real[0, core * nc_loc + cl], w_imag[0, core * nc_loc + cl])
        effs.append(eff)
        effT = eff.T                        # [256(kw), 129(kh)]
        for k, arr in enumerate([effT.real, effT.imag, -effT.imag]):
            a32 = np.zeros((W, KP), np.float32)
            a32[:, 0:KHF] = arr.astype(np.float32)
            warr[cl, k] = np.concatenate([a32, a32], axis=1)
    return warr, effs


def _prep_core_inputs(x, w_real, w_imag, core):
    cs = slice(core * NC_LOC, (core + 1) * NC_LOC)
    xc = np.ascontiguousarray(x[:, cs].transpose(1, 0, 2, 3)).reshape(
        B * NC_LOC, H, W).astype(np.float32)
    warr, _ = _prep_weights(w_real, w_imag, core)
    return {"x": xc, "w": warr}


_NC_CACHE = {}


def kernel(x, w_real, w_imag):
    from concourse.bass_utils import run_bass_kernel_spmd
    x = np.asarray(x); w_real = np.asarray(w_real); w_imag = np.asarray(w_imag)
    key = "full"
    if key not in _NC_CACHE:
        _NC_CACHE[key] = build_nc()
    nc = _NC_CACHE[key]
    in_maps = [_prep_core_inputs(x, w_real, w_imag, i) for i in range(N_CORES)]
    res = run_bass_kernel_spmd(nc, in_maps, core_ids=list(range(N_CORES)))
    outs = []
    for i in range(N_CORES):
        yc = res.results[i]["y"].reshape(NC_LOC, B, H, W).transpose(1, 0, 2, 3)
        outs.append(yc)
    return np.concatenate(outs, axis=1)



# revision 3
# speedup vs baseline: 1.7294x; 1.7294x over previous
"""FFT conv block (rfft2 -> per-channel complex multiply -> irfft2) on 8 trn2 cores.

Math (verified vs reference in float64):
  Work per (b,c) image [256, 256], sharded over channels (8 ch/core):
    T1: rfft over H via matmul            Y1[w, kh]  kh in 0..128
    T2: full fft over W via matmul        Y2[kw, kh] kw in 0..255
    wm: Yw = Y2 * W_eff[kw, kh]           (elementwise complex, DVE)
    T3: ifft over W via matmul            Z[kh, w']
    T4: irfft over H via matmul           y[h, w']   real
  W_eff remaps the reference's [kh_full, kw_half] weights onto the
  [kh_half, kw_full] quarter-plane (conj-flip for kw>128; kw in {0,128}
  columns symmetrized over kh).  Matmuls run in float32r (~1.5e-4 rel).
"""
import sys
sys.path.insert(0, "/opt/trn_rl_repo")
import numpy as np

B, C, H, W = 16, 64, 256, 256
KHF = H // 2 + 1          # 129
N_CORES = 8
NC_LOC = C // N_CORES     # 8 channels per core
NB = B                    # 16 batch images per channel
G = 8                     # supergroup size (images of same channel)


def _consts():
    h = np.arange(H)[:, None]
    kh = np.arange(KHF)[None, :]
    th = 2 * np.pi * h * kh / H                      # [H, KHF]
    z = np.zeros((H, 1))
    M1 = np.concatenate([np.cos(th), z, -np.sin(th), z], axis=1)  # [256, 260]

    w_ = np.arange(W)[:, None]
    kw = np.arange(W)[None, :]
    tw = 2 * np.pi * w_ * kw / W                     # [W, W]
    P = np.cos(tw)                                   # [256, 256]
    Q = np.sin(tw)

    kw2 = np.arange(W)[:, None]
    wp = np.arange(W)[None, :]
    t3 = 2 * np.pi * kw2 * wp / W
    C3 = np.cos(t3) / W                              # [256, 256]
    S3 = np.sin(t3) / W

    kh2 = np.arange(KHF)[:, None]
    hp = np.arange(H)[None, :]
    t4 = 2 * np.pi * kh2 * hp / H
    alpha = np.where((kh2 == 0) | (kh2 == H // 2), 1.0, 2.0)
    A4 = alpha * np.cos(t4) / H                      # [129, 256]
    B4 = -alpha * np.sin(t4) / H                     # [129, 256] (rows 0,128 zero)
    B4[0, :] = A4[128, :]   # row 0 slot carries the nyquist contribution via Zi[0]
    f32 = np.float32
    return (M1.astype(f32), P.astype(f32), Q.astype(f32), C3.astype(f32),
            S3.astype(f32), A4.astype(f32), B4.astype(f32))


def _w_eff(wr, wi):
    """wr, wi: [256(kh), 129(kw)] reference layout -> W_eff [129(kh), 256(kw)]."""
    w = wr.astype(np.float64) + 1j * wi.astype(np.float64)
    kh = np.arange(KHF)                              # 0..128
    khc = (H - kh) % H
    eff = np.empty((KHF, W), dtype=np.complex128)
    eff[:, 0:W // 2 + 1] = w[0:KHF, :]
    for kw in range(W // 2 + 1, W):
        eff[:, kw] = np.conj(w[khc, W - kw])
    for col in (0, W // 2):
        eff[:, col] = 0.5 * (w[kh, col] + np.conj(w[khc, col]))
    return eff                                        # [129, 256] complex128


def _np_pipeline(x_img, eff):
    """Golden single-image model (float64) used by test harness."""
    Q_ = np.fft.fft2(x_img.astype(np.float64))[0:KHF, :]
    Yq = Q_ * eff
    return np.fft.irfft(np.fft.ifft(Yq, axis=1), n=H, axis=0)


def build_nc(nc_loc=NC_LOC, nb=NB, g=G, repeat=1):
    import concourse.bass as bass
    import concourse.mybir as mybir
    import concourse.tile as tile
    from concourse import bacc

    f32, f32r = mybir.dt.float32, mybir.dt.float32r
    bf16 = mybir.dt.bfloat16
    KP = 130                               # kh padded to even (fp32r ISA rule)
    n_img = nb * nc_loc

    M1, P, Q, C3, S3, A4, B4 = _consts()

    nc = bacc.Bacc("TRN2", target_bir_lowering=False)
    # Host-transposed layouts: partition dim first so every DMA line is one
    # large contiguous descriptor per partition (16-50KB) instead of 1KB.
    x_d = nc.dram_tensor("x", [128, n_img, 2, W], f32r, kind="ExternalInput")
    w_d = nc.dram_tensor("w", [128, nc_loc, 3, 2, 2, KP], f32,
                         kind="ExternalInput")
    y_d = nc.dram_tensor("y", [128, n_img, 2, W], f32, kind="ExternalOutput")

    c_m1 = nc.inline_tensor(M1, "c_m1")            # [256, 260]
    c_p = nc.inline_tensor(P, "c_p")               # [256, 256]
    c_q = nc.inline_tensor(Q, "c_q")
    c_nq = nc.inline_tensor(-Q, "c_nq")
    c_c3 = nc.inline_tensor(C3, "c_c3")
    c_s3 = nc.inline_tensor(S3, "c_s3")
    c_ns3 = nc.inline_tensor(-S3, "c_ns3")
    c_a4m = nc.inline_tensor(A4[0:128], "c_a4m")   # [128, 256]
    c_b4m = nc.inline_tensor(B4[0:128], "c_b4m")   # [128, 256]

    from contextlib import ExitStack
    with tile.TileContext(nc) as tc, ExitStack() as es:
        cpool = es.enter_context(tc.tile_pool(name="const", bufs=1))
        stage = es.enter_context(tc.tile_pool(name="stage", bufs=2))
        wpool = es.enter_context(tc.tile_pool(name="wpool", bufs=1))
        xrp = es.enter_context(tc.tile_pool(name="xr", bufs=2))
        y1p = es.enter_context(tc.tile_pool(name="y1", bufs=4))
        y2wp = es.enter_context(tc.tile_pool(name="y2w", bufs=2))
        wtmp = es.enter_context(tc.tile_pool(name="wtmp", bufs=2))
        nyqbp = es.enter_context(tc.tile_pool(name="nyqb", bufs=2))
        zsb = es.enter_context(tc.tile_pool(name="zsb", bufs=4))
        znsb = es.enter_context(tc.tile_pool(name="znsb", bufs=2))
        ysb = es.enter_context(tc.tile_pool(name="ysb", bufs=1))
        ps1 = es.enter_context(tc.tile_pool(name="ps1", bufs=2, space="PSUM"))
        ps2 = es.enter_context(tc.tile_pool(name="ps2", bufs=2, space="PSUM"))
        ps3 = es.enter_context(tc.tile_pool(name="ps3", bufs=2, space="PSUM"))
        ps4 = es.enter_context(tc.tile_pool(name="ps4", bufs=2, space="PSUM"))
        if True:

            def load_const(src, rows, cols, nm, dt_):
                tiles = []
                nch = (rows + 127) // 128
                for ch in range(nch):
                    r0, r1 = ch * 128, min((ch + 1) * 128, rows)
                    st = stage.tile([128, 520], f32, name="st")[0:r1 - r0, 0:cols]
                    nc.sync.dma_start(out=st, in_=src[r0:r1, :])
                    t = cpool.tile([r1 - r0, cols], dt_, name=f"c_{nm}{ch}")
                    nc.vector.tensor_copy(out=t, in_=st)
                    tiles.append(t)
                return tiles

            m1t = load_const(c_m1, H, 2 * KP, "m1", f32r)      # 2 x [128, 260]
            pt = load_const(c_p, W, W, "p", f32r)              # 2 x [128, 256]
            qt = load_const(c_q, W, W, "q", f32r)
            nqt = load_const(c_nq, W, W, "nq", f32r)
            c3t = load_const(c_c3, W, W, "c3", f32r)
            s3t = load_const(c_s3, W, W, "s3", f32r)
            ns3t = load_const(c_ns3, W, W, "ns3", f32r)
            a4mt = load_const(c_a4m, 128, W, "a4m", f32r)[0]
            b4mt = load_const(c_b4m, 128, W, "b4m", f32r)[0]
            c3tb = load_const(c_c3, W, W, "c3b", bf16)         # bf16 for nyquist mm
            ns3tb = load_const(c_ns3, W, W, "ns3b", bf16)

            # weights: one big tile, single DMA (contiguous ~50KB per partition)
            w_big = wpool.tile([128, nc_loc, 3, 2, 2, KP], f32, name="w_big")
            nc.sync.dma_start(out=w_big, in_=w_d[0:128])
            wt = [[w_big[:, cl, k] for k in range(3)] for cl in range(nc_loc)]

            npairs = g // 2
            for _rep in range(repeat):
              for cl in range(nc_loc):
                for sg0 in range(0, nb, g):
                    imgb = cl * nb + sg0
                    # ---- phase A: T1 + T2 + wmul per pair ----
                    xr_sg = xrp.tile([128, g, 2, W], f32r, name="xr_sg")
                    nc.sync.dma_start(out=xr_sg, in_=x_d[:, imgb:imgb + g])
                    y2r_sg = y2wp.tile([128, 2, g, KP], f32r, name="y2r_sg")
                    y2i_sg = y2wp.tile([128, 2, g, KP], f32r, name="y2i_sg")
                    for pr in range(npairs):
                        y1t = [y1p.tile([128, 2, 2 * KP], f32r, name=f"y1_{wc}")
                               for wc in range(2)]
                        for j in range(2):
                            sj = 2 * pr + j
                            for wc in range(2):
                                t1 = ps1.tile([128, 2 * KP], f32, name="t1ps")
                                nc.tensor.matmul(
                                    t1, xr_sg[:, sj, 0, wc * 128:(wc + 1) * 128],
                                    m1t[0], start=True, stop=False)
                                nc.tensor.matmul(
                                    t1, xr_sg[:, sj, 1, wc * 128:(wc + 1) * 128],
                                    m1t[1], start=False, stop=True)
                                nc.scalar.copy(out=y1t[wc][:, j, :], in_=t1)
                        # T2: Y2r = P^T Y1r + Q^T Y1i ; Y2i = P^T Y1i - Q^T Y1r
                        for kwc in range(2):
                            cols = slice(kwc * 128, (kwc + 1) * 128)
                            y2r = ps2.tile([128, 2, KP], f32, name="y2rps", bufs=1)
                            y2i = ps2.tile([128, 2, KP], f32, name="y2ips", bufs=1)
                            r_, i_ = slice(0, KP), slice(KP, 2 * KP)
                            nc.tensor.matmul(y2r, pt[0][:, cols], y1t[0][:, :, r_], start=True, stop=False)
                            nc.tensor.matmul(y2r, pt[1][:, cols], y1t[1][:, :, r_], start=False, stop=False)
                            nc.tensor.matmul(y2r, qt[0][:, cols], y1t[0][:, :, i_], start=False, stop=False)
                            nc.tensor.matmul(y2r, qt[1][:, cols], y1t[1][:, :, i_], start=False, stop=True)
                            nc.tensor.matmul(y2i, pt[0][:, cols], y1t[0][:, :, i_], start=True, stop=False)
                            nc.tensor.matmul(y2i, pt[1][:, cols], y1t[1][:, :, i_], start=False, stop=False)
                            nc.tensor.matmul(y2i, nqt[0][:, cols], y1t[0][:, :, r_], start=False, stop=False)
                            nc.tensor.matmul(y2i, nqt[1][:, cols], y1t[1][:, :, r_], start=False, stop=True)
                            # wmul: Ywr = Y2r*Wr + Y2i*(-Wi); Ywi = Y2r*Wi + Y2i*Wr
                            wr_ = wt[cl][0][:, kwc, :, :]
                            wi_ = wt[cl][1][:, kwc, :, :]
                            nwi = wt[cl][2][:, kwc, :, :]
                            sl = slice(2 * pr, 2 * pr + 2)
                            ta = wtmp.tile([128, 2, KP], f32, name="ta")
                            tb = wtmp.tile([128, 2, KP], f32, name="tb")
                            nc.vector.tensor_mul(ta, y2r, wr_)
                            nc.vector.tensor_mul(tb, y2i, nwi)
                            nc.vector.tensor_add(y2r_sg[:, kwc, sl, :], ta, tb)
                            tc_ = wtmp.tile([128, 2, KP], f32, name="tc")
                            td = wtmp.tile([128, 2, KP], f32, name="td")
                            nc.vector.tensor_mul(tc_, y2r, wi_)
                            nc.vector.tensor_mul(td, y2i, wr_)
                            nc.vector.tensor_add(y2i_sg[:, kwc, sl, :], tc_, td)
                    # ---- phase B: nyquist (kh=128) T3, real part only, bf16 mm ----
                    nbr = nyqbp.tile([128, 2, g], bf16, name="nbr")
                    nbi = nyqbp.tile([128, 2, g], bf16, name="nbi")
                    nc.vector.tensor_copy(out=nbr, in_=y2r_sg[:, :, :, 128])
                    nc.vector.tensor_copy(out=nbi, in_=y2i_sg[:, :, :, 128])
                    zn = ps1.tile([g, W], f32, name="znps", tag="t1ps")
                    nc.tensor.matmul(zn, nbr[:, 0, :], c3tb[0], start=True, stop=False)
                    nc.tensor.matmul(zn, nbr[:, 1, :], c3tb[1], start=False, stop=False)
                    nc.tensor.matmul(zn, nbi[:, 0, :], ns3tb[0], start=False, stop=False)
                    nc.tensor.matmul(zn, nbi[:, 1, :], ns3tb[1], start=False, stop=True)
                    znr = znsb.tile([g, W], f32r, name="znr")
                    nc.vector.tensor_copy(out=znr, in_=zn)
                    # ---- phase C: T3 for all g images, then nyquist row DMA,
                    # then T4 for all g images, one batched y store ----
                    zrs_sg = zsb.tile([128, g, W], f32r, name="zrs_sg", bufs=1)
                    zis_sg = zsb.tile([128, g, W], f32r, name="zis_sg", bufs=1)
                    for gg in range(g):
                        zr = ps3.tile([128, W], f32, name="zrps", bufs=1)
                        zi = ps3.tile([128, W], f32, name="zips", bufs=1)
                        kmain = slice(0, 128)
                        nc.tensor.matmul(zr, y2r_sg[:, 0, gg, kmain], c3t[0], start=True, stop=False)
                        nc.tensor.matmul(zr, y2r_sg[:, 1, gg, kmain], c3t[1], start=False, stop=False)
                        nc.tensor.matmul(zr, y2i_sg[:, 0, gg, kmain], ns3t[0], start=False, stop=False)
                        nc.tensor.matmul(zr, y2i_sg[:, 1, gg, kmain], ns3t[1], start=False, stop=True)
                        nc.tensor.matmul(zi, y2i_sg[:, 0, gg, kmain], c3t[0], start=True, stop=False)
                        nc.tensor.matmul(zi, y2i_sg[:, 1, gg, kmain], c3t[1], start=False, stop=False)
                        nc.tensor.matmul(zi, y2r_sg[:, 0, gg, kmain], s3t[0], start=False, stop=False)
                        nc.tensor.matmul(zi, y2r_sg[:, 1, gg, kmain], s3t[1], start=False, stop=True)
                        nc.scalar.copy(out=zrs_sg[:, gg, :], in_=zr)
                        nc.vector.tensor_copy(out=zis_sg[:, gg, :], in_=zi)
                    nc.sync.dma_start(out=zis_sg[0:1, :, :], in_=znr)
                    yt_sg = ysb.tile([128, g, 2, W], f32, name="yt_sg")
                    for gg in range(g):
                        for hc in range(2):
                            cols = slice(hc * 128, (hc + 1) * 128)
                            yp = ps4.tile([128, W], f32, name="yps")
                            nc.tensor.matmul(yp, a4mt[:, cols], zrs_sg[:, gg, :], start=True, stop=False)
                            nc.tensor.matmul(yp, b4mt[:, cols], zis_sg[:, gg, :], start=False, stop=True)
                            nc.scalar.copy(out=yt_sg[:, gg, hc, :], in_=yp)
                    nc.sync.dma_start(out=y_d[:, imgb:imgb + g], in_=yt_sg)
    nc.compile()
    return nc


def _prep_weights(w_real, w_imag, core, nc_loc=NC_LOC):
    KP = 130
    # layout [128(p), nc_loc, 3, 2(kw-half), 2(imgdup), KP]
    warr = np.zeros((128, nc_loc, 3, 2, 2, KP), np.float32)
    effs = []
    for cl in range(nc_loc):
        eff = _w_eff(w_real[0, core * nc_loc + cl], w_imag[0, core * nc_loc + cl])
        effs.append(eff)
        effT = eff.T                        # [256(kw), 129(kh)]
        for k, arr in enumerate([effT.real, effT.imag, -effT.imag]):
            a32 = np.zeros((W, KP), np.float32)
            a32[:, 0:KHF] = arr.astype(np.float32)
            ach = a32.reshape(2, 128, KP).transpose(1, 0, 2)  # [128, ch, KP]
            warr[:, cl, k, :, 0, :] = ach
            warr[:, cl, k, :, 1, :] = ach
    return warr, effs


def _prep_core_inputs(x, w_real, w_imag, core):
    cs = slice(core * NC_LOC, (core + 1) * NC_LOC)
    # device layout [128(p), n_img=(cl, b), 2(h-half), W] in one permute
    xt = np.ascontiguousarray(
        x[:, cs].reshape(B, NC_LOC, 2, 128, W).transpose(3, 1, 0, 2, 4)
        .reshape(128, B * NC_LOC, 2, W).astype(np.float32))
    warr, _ = _prep_weights(w_real, w_imag, core)
    return {"x": xt, "w": warr}


_NC_CACHE = {}


def kernel(x, w_real, w_imag):
    from concourse.bass_utils import run_bass_kernel_spmd
    x = np.asarray(x); w_real = np.asarray(w_real); w_imag = np.asarray(w_imag)
    key = "full"
    if key not in _NC_CACHE:
        _NC_CACHE[key] = build_nc()
    nc = _NC_CACHE[key]
    in_maps = [_prep_core_inputs(x, w_real, w_imag, i) for i in range(N_CORES)]
    res = run_bass_kernel_spmd(nc, in_maps, core_ids=list(range(N_CORES)))
    outs = []
    for i in range(N_CORES):
        yt = res.results[i]["y"]            # [128, n_img, 2, W]
        yc = yt.transpose(1, 2, 0, 3).reshape(NC_LOC, B, H, W).transpose(
            1, 0, 2, 3)
        outs.append(yc)
    return np.concatenate(outs, axis=1)

